# revision 1
# baseline (speedup 1.0000x reference)
"""DotLinkPredictor v2: SBUF-resident bf16 node table + SBUF-source gathers.

score[e] = dot(h[src[e]], h[dst[e]]);  E=1M edges, h [100000, 64] f32.

Design (vs the HBM-gather baseline): the baseline's dma_gather descriptors
each pay a random-HBM round trip (~800ns/engine serial -> ~51ns/desc
aggregate -> 12.8ms). Here h is converted to bf16 and packed into SBUF
(12.8MB), and gathers use the SBUF-source transpose mode of dma_gather,
which avoids the HBM round trip per descriptor.

Layout:
  - bf16 rows are 128B; dma_gather requires 256B elements, so NODE PAIRS
    (2k, 2k+1) are packed per token. An edge endpoint's parity selects the
    low/high 64 partitions of the transposed gather output.
  - int16 indices limit a gather to 32768 tokens -> 2 chunks of 25000 pairs.
  - Edges are grouped host-side into 16 segments by
    (chunk_u, parity_u, chunk_v, parity_v); within a segment the gather
    source view and partition slices are uniform.
  - Transpose gather output: [128 partitions = bf16 feature slots, N cols =
    edges]. Per segment: DVE in-place mult of the two 64-partition slices,
    then a log2 partition-halving add tree; final add writes scores to
    partition s of the sc tile. Scores return as bf16 [16, cap] per core.
"""

import numpy as np

import concourse.bacc as bacc
import concourse.mybir as mybir

N_NODES = 100000
D = 64
N_EDGES = 1000000
N_CORES = 8
EPC = N_EDGES // N_CORES
P = 128
NCHUNK = 2
NODES_PER_CHUNK = 50000
PAIRS_PER_CHUNK = 25000
RANKS = 196                     # ceil(25000 / 128)
CHUNK_COLS = RANKS * 128        # int16 cols per chunk stripe
HP_COLS = NCHUNK * CHUNK_COLS   # 50176
NSEG = 16
MAX_SUB = 1024
NIDXBUF = 4                     # index-slice ring depth

_PROG_CACHE = {}


def _to_bf16_u16(x):
    """f32 ndarray -> bf16 bit pattern (uint16), round-to-nearest-even."""
    u = np.ascontiguousarray(x, dtype=np.float32).view(np.uint32)
    r = ((u >> 16) & 1) + np.uint32(0x7FFF)
    return ((u + r) >> 16).astype(np.uint16)


def _bf16_to_f32(u16):
    return (u16.astype(np.uint32) << 16).view(np.float32)


def _pack_h(h):
    """h [100000, 64] f32 -> [128, HP_COLS] int16 pair-packed SBUF image.

    Pair p of chunk c sits at partition (p%25000... local)&127, rank
    local>>7, cols [c*CHUNK_COLS + rank*128, +128): 64 cols even node
    features, 64 cols odd node features (bf16 bits)."""
    hb = _to_bf16_u16(h)                       # [100000, 64] u16
    pairs = hb.reshape(50000, 128)             # [pair, 128] = even||odd
    img = np.zeros((NCHUNK, RANKS * 128, 128), dtype=np.uint16)
    img[0, :PAIRS_PER_CHUNK] = pairs[:PAIRS_PER_CHUNK]
    img[1, :PAIRS_PER_CHUNK] = pairs[PAIRS_PER_CHUNK:]
    # [chunk, rank, tok, col] -> [tok, chunk, rank, col]
    img = img.reshape(NCHUNK, RANKS, 128, 128).transpose(2, 0, 1, 3)
    return np.ascontiguousarray(img.reshape(128, HP_COLS)).view(np.int16)


def _wrap16(padded, cap):
    """[NSEG, cap] int16 -> [128, NSEG*cap//16] dma_gather wrapped layout."""
    idxc = cap // 16
    w = padded.reshape(NSEG, idxc, 16).transpose(0, 2, 1)
    w = np.tile(w, (1, P // 16, 1))
    return np.ascontiguousarray(w.transpose(1, 0, 2).reshape(P, NSEG * idxc))


def _subs_for(cap):
    subs = [MAX_SUB] * (cap // MAX_SUB)
    if cap % MAX_SUB:
        subs.append(cap % MAX_SUB)
    return subs


def _seg_decode(s):
    return (s >> 3) & 1, (s >> 2) & 1, (s >> 1) & 1, s & 1  # cu, pu, cv, pv


def build(cap, n_queues=4, u_queues=(0, 2), v_queues=(1, 3), repeat=1,
          do_gather=True, do_compute=True, sub=512, depth=1):
    """repeat>1 re-runs the whole segment pipeline on-device (same data,
    cumulative semaphore targets) so per-iteration time can be measured as a
    slope, immune to dispatch jitter. do_gather/do_compute isolate the two
    halves for bottleneck attribution (outputs are garbage when either is
    False)."""
    assert cap % sub == 0
    # HW constraint: multi-packet transpose gathers (>512 idxs, or any with
    # single_packet=False) corrupt each other when interleaved across SWDGE
    # queues. sub<=512 runs single-packet (safe on any queues); larger subs
    # force every gather onto one queue.
    single_packet = sub <= 512
    if not single_packet:
        u_queues = v_queues = (0,)
    idxc = cap // 16
    subs = [sub] * (cap // sub)
    nsub = len(subs)
    NG = NSEG * repeat
    nc = bacc.Bacc("TRN2", target_bir_lowering=False, debug=False,
                   num_swdge_queues=n_queues)
    hp_t = nc.dram_tensor("hp", [P, HP_COLS], mybir.dt.int16,
                          kind="ExternalInput")
    su_t = nc.dram_tensor("su", [P, NSEG * idxc], mybir.dt.int16,
                          kind="ExternalInput")
    sv_t = nc.dram_tensor("sv", [P, NSEG * idxc], mybir.dt.int16,
                          kind="ExternalInput")
    cs = cap // 32
    out_t = nc.dram_tensor("scores", [32, NSEG * cs], mybir.dt.bfloat16,
                           kind="ExternalOutput")

    hp_s = nc.alloc_sbuf_tensor("hp_s", [P, HP_COLS], mybir.dt.int16)
    su_s = nc.alloc_sbuf_tensor("su_s", [P, NSEG * idxc], mybir.dt.int16)
    sv_s = nc.alloc_sbuf_tensor("sv_s", [P, NSEG * idxc], mybir.dt.int16)
    nbuf = min(depth, 2)
    gu = [nc.alloc_sbuf_tensor(f"gu{i}", [P, 1, cap], mybir.dt.bfloat16)
          for i in range(nbuf)]
    gv = [nc.alloc_sbuf_tensor(f"gv{i}", [P, 1, cap], mybir.dt.bfloat16)
          for i in range(nbuf)]
    sc = nc.alloc_sbuf_tensor("sc", [32, NSEG * cs], mybir.dt.bfloat16)

    import contextlib
    with contextlib.ExitStack() as stack:
        block = stack.enter_context(nc.Block())
        hp_sem = stack.enter_context(nc.semaphore("hp_sem"))
        zs_sem = stack.enter_context(nc.semaphore("zs_sem"))
        out_sem = stack.enter_context(nc.semaphore("out_sem"))
        idx_sem = stack.enter_context(nc.semaphore("idx_sem"))
        gq_sems = [stack.enter_context(nc.semaphore(f"gq{q}"))
                   for q in range(n_queues)]
        mul_sems = [stack.enter_context(nc.semaphore(f"mul{s}"))
                    for s in range(NSEG)]
        comp_sems = [stack.enter_context(nc.semaphore(f"cmp{s}"))
                     for s in range(NSEG)]

        @block.sync
        def _(s):
            s.dma_start(out=hp_s[:], in_=hp_t[:]).then_inc(hp_sem, 16)
            s.dma_start(out=su_s[:], in_=su_t[:]).then_inc(idx_sem, 16)
            s.dma_start(out=sv_s[:], in_=sv_t[:]).then_inc(idx_sem, 16)
            for i in range(NSEG):
                s.wait_ge(comp_sems[i], repeat)
            for i in range(NSEG):
                s.dma_start(out=out_t[:, i * cs:(i + 1) * cs],
                            in_=sc[:, i * cs:(i + 1) * cs]
                            ).then_inc(out_sem, 16)
            s.wait_ge(out_sem, 16 * NSEG)

        qcount = [0] * n_queues
        qdone = [0] * n_queues
        qtargets = []

        @block.gpsimd
        def _(g):
            g.wait_ge(zs_sem, nbuf * 2)
            g.wait_ge(hp_sem, 16)
            g.wait_ge(idx_sem, 32)
            for gg in range(NG):
                s, r = gg % NSEG, gg // NSEG
                cu, pu, cv, pv = _seg_decode(s)
                if gg >= depth:
                    # gu free after the reduce; gv doubles as the
                    # transpose target so it is also held until then.
                    # depth=1 fully serializes gathers vs DVE compute.
                    p = gg - depth
                    g.wait_ge(comp_sems[p % NSEG], p // NSEG + 1)
                ub, vb = gu[gg % nbuf], gv[gg % nbuf]
                in_u = hp_s[:, cu * CHUNK_COLS:(cu + 1) * CHUNK_COLS]
                in_v = hp_s[:, cv * CHUNK_COLS:(cv + 1) * CHUNK_COLS]
                if not do_gather:
                    for gi in range(nsub):
                        qcount[u_queues[gi % len(u_queues)]] += 16
                        qcount[v_queues[gi % len(v_queues)]] += 16
                    for q in range(n_queues):
                        if qcount[q] > qdone[q]:
                            g.sem_inc(gq_sems[q], qcount[q] - qdone[q])
                            qdone[q] = qcount[q]
                    qtargets.append(tuple(qcount))
                    continue
                off = 0
                for gi, nidx in enumerate(subs):
                    c0 = off // 16
                    icols = nidx // 16
                    uq = u_queues[gi % len(u_queues)]
                    vq = v_queues[gi % len(v_queues)]
                    g.dma_gather(
                        out_ap=ub[:, :, off:off + nidx], in_ap=in_u,
                        idxs_ap=su_s[:, s * idxc + c0:s * idxc + c0 + icols],
                        num_idxs=nidx, num_idxs_reg=nidx, elem_size=128,
                        transpose=True,
                        sbuf_tokens_per_rank=128,
                        sbuf_free_dim_per_rank=256,
                        single_packet=single_packet,
                        queue_num=uq,
                    ).then_inc(gq_sems[uq], 16)
                    qcount[uq] += 16
                    g.dma_gather(
                        out_ap=vb[:, :, off:off + nidx], in_ap=in_v,
                        idxs_ap=sv_s[:, s * idxc + c0:s * idxc + c0 + icols],
                        num_idxs=nidx, num_idxs_reg=nidx, elem_size=128,
                        transpose=True,
                        sbuf_tokens_per_rank=128,
                        sbuf_free_dim_per_rank=256,
                        single_packet=single_packet,
                        queue_num=vq,
                    ).then_inc(gq_sems[vq], 16)
                    qcount[vq] += 16
                    off += nidx
                qtargets.append(tuple(qcount))

        @block.vector
        def _(v):
            for b in gu + gv:
                v.memset(b[:], 0.0).then_inc(zs_sem, 1)
            nstep = [0] * NSEG
            for gg in range(NG):
                s, r = gg % NSEG, gg // NSEG
                cu, pu, cv, pv = _seg_decode(s)
                for q in range(n_queues):
                    if qtargets[gg][q]:
                        v.wait_ge(gq_sems[q], qtargets[gg][q])
                ub, vb = gu[gg % nbuf], gv[gg % nbuf]
                bu = 64 * pu
                bv = 64 * pv
                if not do_compute:
                    v.sem_inc(comp_sems[s], 1)
                    continue

                def step(inst):
                    nstep[s] += 1
                    inst.then_inc(mul_sems[s], 1)
                    v.wait_ge(mul_sems[s], nstep[s])

                # tensor_tensor inputs must share a base partition; for
                # mixed parity, rebase-copy v's live half over its dead half
                if pu != pv:
                    step(v.tensor_copy(
                        vb[bu:bu + 64, 0, :], vb[bv:bv + 64, 0, :]))
                step(v.tensor_tensor(
                    out=ub[bu:bu + 64, 0, :], in0=ub[bu:bu + 64, 0, :],
                    in1=vb[bu:bu + 64, 0, :], op=mybir.AluOpType.mult))
                # fold 64->32: rebase-copy the high half next to the low
                # half, add with output rebased to partition 0
                step(v.tensor_copy(
                    vb[bu:bu + 32, 0, :], ub[bu + 32:bu + 64, 0, :]))
                step(v.tensor_tensor(
                    out=ub[0:32, 0, :], in0=ub[bu:bu + 32, 0, :],
                    in1=vb[bu:bu + 32, 0, :], op=mybir.AluOpType.add))
                # 32x32 block transpose, then a free-axis windowed reduce:
                # scores for edge r land at partition r%32, col s*cs + r//32
                step(v.transpose(out=vb[0:32, 0, :], in_=ub[0:32, 0, :]))
                with nc.allow_low_precision("f32 accum, bf16 out"):
                    v.tensor_reduce(
                        out=sc[:, s * cs:(s + 1) * cs],
                        in_=vb[0:32, 0, :].rearrange("p (b w) -> p b w", w=32),
                        axis=mybir.AxisListType.X, op=mybir.AluOpType.add,
                    ).then_inc(comp_sems[s], 1)

    nc.compile()
    return nc


def get_prog(cap, **kw):
    key = (cap, tuple(sorted(kw.items())))
    if key not in _PROG_CACHE:
        _PROG_CACHE[key] = build(cap, **kw)
    return _PROG_CACHE[key]


def prepare(h, src, dst):
    """Host prep: pack h, globally balance edges across cores per segment,
    build fully-valid (0-padded) wrapped index windows.

    Edges are assigned to cores by dealing each global segment's edges
    round-robin, so per-core segment sizes are within 1 of the global
    average -> minimal cap and perfectly balanced cores."""
    hp = _pack_h(np.asarray(h))
    src = np.asarray(src).astype(np.int64)
    dst = np.asarray(dst).astype(np.int64)

    cu = src // NODES_PER_CHUNK
    pu = src & 1
    cv = dst // NODES_PER_CHUNK
    pv = dst & 1
    key = (cu * 8 + pu * 4 + cv * 2 + pv).astype(np.uint8)
    lu = ((src % NODES_PER_CHUNK) >> 1).astype(np.int16)
    lv = ((dst % NODES_PER_CHUNK) >> 1).astype(np.int16)

    order_all = np.argsort(key, kind="stable")
    counts_all = np.bincount(key, minlength=NSEG)
    starts = np.zeros(NSEG + 1, dtype=np.int64)
    starts[1:] = np.cumsum(counts_all)

    max_per_core = int(np.ceil(counts_all.max() / N_CORES))
    cap = int(np.ceil(max_per_core / MAX_SUB)) * MAX_SUB

    in_maps = []
    recon = []
    for c in range(N_CORES):
        pu_idx = np.zeros((NSEG, cap), dtype=np.int16)
        pv_idx = np.zeros((NSEG, cap), dtype=np.int16)
        eids = np.full((NSEG, cap), -1, dtype=np.int64)
        for s in range(NSEG):
            ids = order_all[starts[s]:starts[s + 1]][c::N_CORES]
            n = len(ids)
            assert n <= cap
            pu_idx[s, :n] = lu[ids]
            pv_idx[s, :n] = lv[ids]
            eids[s, :n] = ids
        in_maps.append({
            "hp": hp,
            "su": _wrap16(pu_idx, cap),
            "sv": _wrap16(pv_idx, cap),
        })
        recon.append(eids)
    return in_maps, recon, cap


def unpack_into(out, res_scores, eids, cap):
    """Scatter one core's [32, NSEG*cap//32] f32 scores into out[E]."""
    cs = cap // 32
    sc = np.asarray(res_scores).astype(np.float32)
    r = np.arange(cap, dtype=np.int64)
    cols = (np.arange(NSEG, dtype=np.int64)[:, None] * cs) + (r[None, :] // 32)
    vals = sc[r[None, :] % 32, cols]          # [NSEG, cap]
    mask = eids >= 0
    out[eids[mask]] = vals[mask]


def kernel(h, src, dst):
    """Full-input entry: shard, run on 8 cores, reassemble."""
    from concourse.bass_utils import run_bass_kernel_spmd

    in_maps, recon, cap = prepare(h, src, dst)
    nc = get_prog(cap)
    res = run_bass_kernel_spmd(nc, in_maps, list(range(N_CORES)))
    out = np.empty(N_EDGES, dtype=np.float32)
    for c in range(N_CORES):
        unpack_into(out, res.results[c]["scores"], recon[c], cap)
    return out

